# revision 13
# baseline (speedup 1.0000x reference)
"""9x9 morphological dilation (sliding-window max, SAME padding) on Trainium2.

Input : label (16, 1024, 1024, 1) float32, values in [0, 1).
Output: same shape; out[b,i,j] = max over the 9x9 window centered at (i,j),
        clipped to the image (cv2-style border handling for dilate).

Strategy (per NeuronCore; batch is data-parallel over 8 cores, 2 images/core):
  - fp16 datapath: inputs are converted f32->fp16 on the ACT engine right
    after load; all max passes run in fp16 (DVE gets the 2x_1p perf mode;
    rel. rounding error ~2^-10 << the 2e-2 tolerance).  Zero is a valid
    -inf substitute for padding since inputs are >= 0.
  - SBUF layout: 128 partitions = 2 images x 64 row-blocks.  Partition q
    holds image rows 16q-4 .. 16q+11 (shifted by the window radius), so
    R9[q,r] = max over rows (16q+r-4)..(16q+r+4) = output row 16q+r: output
    rows align exactly with partitions and each image chunk stores with ONE
    contiguous DMA.  q=0's four pad rows are zero-filled; rows 1020..1023
    ride a small side tile into q=63's halo.
  - Vertical 9-max: each partition's fp16 tile is extended to 24 rows by one
    SBUF->SBUF DMA per image (rows 0..7 of partition p+1 -> rows 16..23 of
    p; full-width so each partition is one 3.8KB descriptor), then a fully
    local, single-engine (DVE) log tree: T2/T4 (+1,+2 row shifts), T8 (+4),
    R9 = max(T8[r], XH[r+8]).
  - Horizontal 9-max: log tree along the free dim.  G2/G4 on DVE; the two
    tail passes (G8 and the f32-writing merge) are offloaded via
    max(a,b) = b + relu(a-b): subtract/add on GpSimd, relu on ACT -- the
    only engines besides DVE that can combine two tensors elementwise.
  - Chunks [128, 232, 232, 232, 200]: narrow first chunk shortens the
    startup chain, narrow last shortens the drain.  Loads are prefetched
    two chunks ahead (3 X slabs) and split across the ACT/SP queues.
"""

import numpy as np

B, H, W = 16, 1024, 1024
NCORES = 8
IMGS = 2            # images per core
RB = 16             # rows per partition block
HALO = 8            # vertical halo rows (window 9 -> 8)
SH = 4              # row shift (window radius)
CHUNKS = [128, 232, 232, 232, 200]   # output cols per chunk (sum = 1024)
assert sum(CHUNKS) == W

_CACHE = {}


def _build(reps=1):
    import concourse.bacc as bacc
    import concourse.tile as tile
    import concourse.mybir as mybir

    f32 = mybir.dt.float32
    f16 = mybir.dt.float16
    sub = mybir.AluOpType.subtract
    add = mybir.AluOpType.add

    nc = bacc.Bacc("TRN2", target_bir_lowering=False, debug=False, num_devices=1)
    x = nc.dram_tensor("x", [IMGS, H, W], f32, kind="ExternalInput").ap()
    y = nc.dram_tensor("y", [IMGS, H, W], f32, kind="ExternalOutput").ap()

    # shifted main view: partition q=1..63 <- rows 16q-4 .. 16q+11
    xvs = [x[i][RB - SH:H - SH].rearrange("(q r) c -> q r c", r=RB)
           for i in range(IMGS)]

    nchunk = len(CHUNKS)
    chunk_off = np.cumsum([0] + CHUNKS[:-1]).tolist()
    U = [cw + 2 * SH for cw in CHUNKS]
    UMAX = max(U)

    with tile.TileContext(nc) as tc:
        with (
            tc.tile_pool(name="px", bufs=3) as px,
            tc.tile_pool(name="pxh", bufs=1) as pxh,
            tc.tile_pool(name="pxb", bufs=1) as pxb,
            tc.tile_pool(name="pt2", bufs=1) as pt2,
            tc.tile_pool(name="pt4", bufs=1) as pt4,
            tc.tile_pool(name="pt8", bufs=1) as pt8,
            tc.tile_pool(name="pr9", bufs=2) as pr9,
            tc.tile_pool(name="pg", bufs=2) as pg,
            tc.tile_pool(name="pd", bufs=2) as pd,
            tc.tile_pool(name="pout", bufs=1) as pout,
        ):
            # persistent fp16 input tiles (ping-pong across chunks), fully
            # zeroed once: the full-width halo DMA and the image-bottom pad
            # rows (p=63/127, rows 20..23) always read initialized data.
            xh_tiles = []
            for s in range(2):
                t = pxh.tile([128, (RB + HALO) * UMAX], f16, tag=f"xh{s}")
                t3 = t.rearrange("p (r u) -> p r u", u=UMAX)
                nc.gpsimd.memset(t[:], 0.0)
                xh_tiles.append(t3)
            # persistent f32 side tiles for image rows 1020..1023 (only
            # partitions 63/127 are used; zeroed once for edge-chunk pads)
            xb_tiles = []
            for s in range(2):
                t = pxb.tile([128, SH * UMAX], f32, tag=f"xb{s}")
                t3 = t.rearrange("p (r u) -> p r u", u=UMAX)
                nc.gpsimd.memset(t[:], 0.0)
                xb_tiles.append(t3)

            def emit_load(it):
                ch = it % nchunk
                cw = CHUNKS[ch]
                u = U[ch]
                c0 = chunk_off[ch]
                clo = max(0, c0 - SH)
                chi = min(W, c0 + cw + SH)
                ncols = chi - clo
                ulo = clo - (c0 - SH)
                X = px.tile([128, RB * UMAX], f32, tag="x")
                x3 = X.rearrange("p (r u) -> p r u", u=UMAX)
                xb3 = xb_tiles[it % 2]
                # left/right image-edge pad cols (slabs are recycled, so
                # re-zero on every edge chunk)
                if ulo > 0:
                    nc.vector.memset(x3[:, :, 0:ulo], 0.0)
                    nc.vector.memset(xb3[:, :, 0:ulo], 0.0)
                if ulo + ncols < u:
                    nc.vector.memset(x3[:, :, ulo + ncols:u], 0.0)
                    nc.vector.memset(xb3[:, :, ulo + ncols:u], 0.0)
                for img in range(IMGS):
                    b = 64 * img
                    eng = nc.scalar if img == 0 else nc.sync
                    # image-top pad rows of q=0 (rows -4..-1)
                    nc.vector.memset(x3[b:b + 1, 0:SH, 0:u], 0.0)
                    eng.dma_start(
                        out=x3[b + 1:b + 64, :, ulo:ulo + ncols],
                        in_=xvs[img][:, :, clo:chi],
                    )
                    eng.dma_start(
                        out=x3[b:b + 1, SH:RB, ulo:ulo + ncols],
                        in_=x[img][0:RB - SH, clo:chi],
                    )
                    eng.dma_start(
                        out=xb3[b + 63:b + 64, :, ulo:ulo + ncols],
                        in_=x[img][H - SH:H, clo:chi],
                    )
                return x3

            def emit_cvt(it, x3):
                u = U[it % nchunk]
                xh3 = xh_tiles[it % 2]
                xb3 = xb_tiles[it % 2]
                # split so the halo DMA (reads rows 0..7) starts earlier
                nc.scalar.copy(xh3[:, 0:HALO, 0:u], x3[:, 0:HALO, 0:u])
                nc.scalar.copy(xh3[:, HALO:RB, 0:u], x3[:, HALO:RB, 0:u])
                # image rows 1020..1023 -> q=63's halo rows 16..19.  Spans
                # all 128 partitions (compute APs must start at 0/32/64):
                # only p=63/127 carry real data; the zeros written elsewhere
                # are overwritten by the halo DMA right after.
                nc.scalar.copy(xh3[:, RB:RB + SH, 0:u], xb3[:, :, 0:u])
                return xh3

            def emit_halo(it, xh3):
                for img in range(IMGS):
                    b = 64 * img
                    nc.sync.dma_start(
                        out=xh3[b:b + 63, RB:RB + HALO, :],
                        in_=xh3[b + 1:b + 64, 0:HALO, :],
                    )

            def emit_vtree(it, xh3):
                ch = it % nchunk
                u = U[ch]

                T2 = pt2.tile([128, 22 * UMAX], f16, tag="t2")
                t2 = T2.rearrange("p (r u) -> p r u", u=UMAX)
                nc.vector.tensor_max(t2[:, 0:22, 0:u], xh3[:, 0:22, 0:u], xh3[:, 1:23, 0:u])

                T4 = pt4.tile([128, 20 * UMAX], f16, tag="t4")
                t4 = T4.rearrange("p (r u) -> p r u", u=UMAX)
                nc.vector.tensor_max(t4[:, 0:20, 0:u], t2[:, 0:20, 0:u], t2[:, 2:22, 0:u])

                T8 = pt8.tile([128, RB * UMAX], f16, tag="t8")
                t8 = T8.rearrange("p (r u) -> p r u", u=UMAX)
                nc.vector.tensor_max(t8[:, 0:16, 0:u], t4[:, 0:16, 0:u], t4[:, 4:20, 0:u])

                R9 = pr9.tile([128, RB * UMAX], f16, tag="r9")
                r9 = R9.rearrange("p (r u) -> p r u", u=UMAX)
                nc.vector.tensor_max(r9[:, 0:16, 0:u], t8[:, 0:16, 0:u], xh3[:, 8:24, 0:u])
                return r9

            def emit_htree(it, r9, last=False):
                ch = it % nchunk
                cw = CHUNKS[ch]
                u = U[ch]
                relu = mybir.ActivationFunctionType.Relu

                G2 = pg.tile([128, RB * UMAX], f16, tag="g")
                g2 = G2.rearrange("p (r u) -> p r u", u=UMAX)
                nc.vector.tensor_max(g2[:, :, 0:u - 2], r9[:, :, 0:u - 2], r9[:, :, 1:u - 1])

                G4 = pg.tile([128, RB * UMAX], f16, tag="g")
                g4 = G4.rearrange("p (r u) -> p r u", u=UMAX)
                nc.vector.tensor_max(g4[:, :, 0:u - 4], g2[:, :, 0:u - 4], g2[:, :, 2:u - 2])

                G8 = pg.tile([128, RB * UMAX], f16, tag="g")
                g8 = G8.rearrange("p (r u) -> p r u", u=UMAX)
                OUT = pout.tile([128, RB * UMAX], f32, tag="out")
                o3 = OUT.rearrange("p (r u) -> p r u", u=UMAX)
                if last:
                    nc.vector.tensor_max(g8[:, :, 0:cw], g4[:, :, 0:cw], g4[:, :, 4:cw + 4])
                    nc.vector.tensor_max(o3[:, :, 0:cw], g8[:, :, 0:cw], r9[:, :, 8:cw + 8])
                    return o3
                # max(a,b) = b + relu(a-b): subtract/add on GpSimd, relu on
                # ACT -- offloads the two tail passes from the DVE
                D8 = pd.tile([128, RB * UMAX], f16, tag="d")
                d8 = D8.rearrange("p (r u) -> p r u", u=UMAX)
                nc.gpsimd.tensor_tensor(
                    d8[:, :, 0:cw], g4[:, :, 0:cw], g4[:, :, 4:cw + 4], op=sub)
                E8 = pd.tile([128, RB * UMAX], f16, tag="d")
                e8 = E8.rearrange("p (r u) -> p r u", u=UMAX)
                nc.scalar.activation(e8[:, :, 0:cw], d8[:, :, 0:cw], relu)
                nc.gpsimd.tensor_tensor(
                    g8[:, :, 0:cw], g4[:, :, 4:cw + 4], e8[:, :, 0:cw], op=add)
                D9 = pd.tile([128, RB * UMAX], f16, tag="d")
                d9 = D9.rearrange("p (r u) -> p r u", u=UMAX)
                nc.gpsimd.tensor_tensor(
                    d9[:, :, 0:cw], g8[:, :, 0:cw], r9[:, :, 8:cw + 8], op=sub)
                E9 = pd.tile([128, RB * UMAX], f16, tag="d")
                e9 = E9.rearrange("p (r u) -> p r u", u=UMAX)
                nc.scalar.activation(e9[:, :, 0:cw], d9[:, :, 0:cw], relu)
                nc.gpsimd.tensor_tensor(
                    o3[:, :, 0:cw], r9[:, :, 8:cw + 8], e9[:, :, 0:cw], op=add)
                return o3

            def emit_stores(it, o3):
                ch = it % nchunk
                cw = CHUNKS[ch]
                c0 = chunk_off[ch]
                for img in range(IMGS):
                    b = 64 * img
                    ymain = y[img][:, c0:c0 + cw].rearrange(
                        "(q r) c -> q r c", r=RB
                    )
                    nc.sync.dma_start(out=ymain, in_=o3[b:b + 64, :, 0:cw])

            # --- software-pipelined emission (loads prefetch 2 ahead) ---
            niter = nchunk * reps
            xp = {0: emit_load(0)}
            xhp = {0: emit_cvt(0, xp.pop(0))}
            emit_halo(0, xhp[0])
            if niter > 1:
                xp[1] = emit_load(1)
            for it in range(niter):
                if it + 2 < niter:
                    xp[it + 2] = emit_load(it + 2)
                if it + 1 < niter:
                    xhp[it + 1] = emit_cvt(it + 1, xp.pop(it + 1))
                    emit_halo(it + 1, xhp[it + 1])
                r9 = emit_vtree(it, xhp.pop(it))
                o3 = emit_htree(it, r9, last=(it == niter - 1))
                emit_stores(it, o3)

    nc.compile()
    return nc


def kernel(label):
    lab = np.ascontiguousarray(
        np.asarray(label, dtype=np.float32).reshape(B, H, W)
    )
    if "nc" not in _CACHE:
        _CACHE["nc"] = _build()
    nc = _CACHE["nc"]

    from concourse.bass_utils import run_bass_kernel_spmd

    in_maps = [{"x": lab[IMGS * c:IMGS * (c + 1)]} for c in range(NCORES)]
    res = run_bass_kernel_spmd(nc, in_maps, core_ids=list(range(NCORES)))
    out = np.concatenate([res.results[c]["y"] for c in range(NCORES)], axis=0)
    return out.reshape(B, H, W, 1)
